# revision 27
# baseline (speedup 1.0000x reference)
"""MoE gate routing kernel for Trainium2 (8 NeuronCores, SPMD token-parallel).

Problem: scores = sigmoid(x @ weight.T); s = scores + bias;
group top-2 sums -> top-4 groups mask -> global top-8 -> gather original
scores -> normalize * 2.5. Returns (w [T,8] f32, idx [T,8] int32).

Exact top-k agreement with the fp32 reference needs fp32-quality scores.
The GEMM runs as a 3-term decomposition at 1 cycle/column each:
    score = xh*wh + xl*wh + xh*wl
where BOTH operands are pre-split ON THE HOST into fp16 hi/lo pairs
(11+11 = 22 mantissa bits each; residuals 2^-22, below the fp32
reference's own einsum summation noise). The splits run in scaled
domains -- x*2^8 and w*2^16 -- so the lo parts stay out of the fp16
subnormal range; the combined 2^24 factor is undone exactly (power of
two) by the sigmoid activation's scale argument. The BIR verifier
requires matmul operand dtypes to match for fp32/f32r, so fp16 x fp16
is the only 1-cycle/column pairing that allows a host-side split.
Benefits vs the previous on-chip f32r-split design: the DVE no longer
spends ~165us splitting x on-chip (it only does routing, which hides
under the PE stream), x DMA bytes are unchanged (2 x fp16 = 1 x fp32),
and the weight DMA halves to 7.3MB.

Structure: weight-stationary [e,t] GEMM with tapered token blocks
[512,512,384,384,256] (the serial tail after the last GEMM is that
block's routing chain, so later blocks shrink), PSUM->SBUF drain copies
on the ACT engine (frees DVE ordering and releases PSUM accumulators
sooner), PE-transpose of score blocks back to [t,e], sigmoid on ACT
from PSUM, routing on DVE with max/max_index/match_replace.

Perf notes (HW-measured, difference-method timing):
  - per fp16 matmul (N=512): ~270ns = 213ns stream (1 col/cyc @2.4GHz)
    + ~55ns serial weight load; the load is NOT hidden by the reorder
    window and NOT skipped for identical back-to-back weights (probed).
  - 3-term GEMM alone = ~365-375us; DMA floor (58.7MB x as fp16 hi/lo
    pairs) = 172us; total = ~385-395us -> PE-bound.
  - matmul N>512 fp32-out fails the walrus ISA check (concat mode is
    dead on HW); striping one matmul's output across two PSUM banks
    does not unlock a faster stream mode (probed).
  - fp8 (e4m3) DoubleRow carrying both corrections in one matmul is
    ~1.45x faster on PE but loses idx-exactness (6/16384 mismatched
    tokens, w relerr 4.6e-2 > 2e-2 gate) -- rejected.
  - ldskip/ldskip2 (dropping standalone InstLdweights) have no effect;
    there is no double weight load on HW.
"""
import sys

if "/opt/trn_rl_repo" not in sys.path:
    sys.path.insert(0, "/opt/trn_rl_repo")

import numpy as np

T, D, E = 16384, 7168, 256
G, KG, KTOP = 8, 4, 8
ROUTE_SCALE = 2.5
NCORES = 8
TCORE = T // NCORES          # 2048 tokens per core
KD = D // 128                # 56 contraction chunks
XSCALE = 256.0               # x pre-scale (2^8): keeps fp16 lo part normal
WSCALE = 65536.0             # w pre-scale (2^16): keeps fp16 lo part normal
SIGSCALE = 1.0 / (256.0 * 65536.0)   # exact 2^-24 descale at the sigmoid
# Token blocks (matmul N <= 512 = PSUM bank). Tapered: the exposed tail
# after the last GEMM block is that block's routing (DVE chain + PE
# transposes), so shrinking the LAST blocks cuts the serial tail, while
# keeping early blocks at 512 for stream efficiency. Measured: uniform
# 512x4 = 399.3us, this taper + ACT-engine drain copies = 384.7us.
# Blocks must be multiples of 128 (routing transposes are 128x128).
BLOCKS = [512, 512, 384, 384, 256]
assert sum(BLOCKS) == TCORE
NT = TCORE // 128            # 16 token tiles per core
KPG = 2                      # k-chunks per DMA
NKG = KD // KPG              # DMAs per block
NWSPLIT = 14                 # weight DMA split (shrinks head bubble)
BIG = 1e30

_CACHE = {}


def _build(bench_iters=0, pipeline=False, nwsplit=NWSPLIT, xbufs=4, blocks=None,
           repeat=1, mode="full", concat=False, trbufs=2, dbuf=False,
           hook_kg=2, ldskip=False, trregion=False, ldextra=0, ldskip2=False,
           terms=3, wpair=False, actcopy=True, qdrain=False, bfgather=False,
           stripe=False, dbuf2=False, kpg=None):
    import concourse.bacc as bacc
    import concourse.mybir as mybir
    import concourse.tile as tile
    from contextlib import ExitStack, nullcontext

    blocks = list(blocks or BLOCKS)
    assert sum(blocks) == TCORE
    KPG = kpg or globals()["KPG"]
    NKG = KD // KPG

    F32 = mybir.dt.float32
    F16 = mybir.dt.float16
    BF16 = mybir.dt.bfloat16
    U32 = mybir.dt.uint32
    X = mybir.AxisListType.X
    Alu = mybir.AluOpType
    Act = mybir.ActivationFunctionType

    nc = bacc.Bacc(None, target_bir_lowering=False, debug=False)

    # x pre-split on host, block-major so every (block, kg) DMA is one
    # contiguous span per partition: per block b the segment holds
    # [KD, {hi,lo}, tb] fp16 flattened
    xt_d = nc.dram_tensor("xt", [128, KD * 2 * TCORE], F16, kind="ExternalInput")
    # weights pre-split into (hi, lo) fp16 parts on the host
    wt_d = nc.dram_tensor("wt", [128, KD * E * 2], F16, kind="ExternalInput")
    bi_d = nc.dram_tensor("bi", [128, 2 * E + 128], F32, kind="ExternalInput")
    w_out_d = nc.dram_tensor("w_out", [128, NT * KTOP], F32, kind="ExternalOutput")
    idx_out_d = nc.dram_tensor("idx_out", [128, NT * KTOP], U32, kind="ExternalOutput")

    with tile.TileContext(nc) as tc, ExitStack() as ctx:
        const = ctx.enter_context(tc.tile_pool(name="const", bufs=1))
        outp = ctx.enter_context(tc.tile_pool(name="outp", bufs=1))
        xpool = ctx.enter_context(tc.tile_pool(name="xp", bufs=xbufs))
        # dbuf: double-buffer the psT accumulators (4 banks) so block b+1's
        # first matmuls never wait on block b's routing copies; psC single
        # (2 banks) + trpool single (2 banks) = 8 banks total.
        pstb = 2 if (dbuf or dbuf2) else 1
        if dbuf:
            trbufs = 1
        if dbuf2:
            dbuf = True
        psTpool = ctx.enter_context(tc.tile_pool(name="psT", bufs=pstb, space="PSUM"))
        pspool = ctx.enter_context(tc.tile_pool(name="ps", bufs=1, space="PSUM"))
        trpool = ctx.enter_context(tc.tile_pool(name="tr", bufs=trbufs, space="PSUM"))
        work = ctx.enter_context(tc.tile_pool(name="work", bufs=2))
        small = ctx.enter_context(tc.tile_pool(name="small", bufs=2))

        # wt_sb[p, k, e_half, hl, e']
        wt_sb = const.tile([128, KD, 2, 2, 128], F16)
        bi_sb = const.tile([128, 2 * E + 128], F32)
        # split the weight load (by k-chunk range) so the first matmuls only
        # wait on a fraction of the 7.3MB weight transfer; issue on the ACT
        # HWDGE queue so x-tile DMAs (SP queue) are not stuck behind it
        ksp = KD // nwsplit
        wt_dv = wt_d[:].rearrange("p (s r) -> p s r", s=nwsplit)
        for sp in range(nwsplit):
            nc.scalar.dma_start(wt_sb[:, sp * ksp:(sp + 1) * ksp], wt_dv[:, sp])
        nc.scalar.dma_start(bi_sb[:], bi_d[:])
        bias_sb = bi_sb[:, 0:E]
        iota_sb = bi_sb[:, E:2 * E]
        ident_sb = bi_sb[:, 2 * E:2 * E + 128]
        iota_bf = None
        if bfgather:
            # bf16 iota (0..255 exact in bf16) for 2x-throughput DVE gathers
            iota_bft = const.tile([128, E], BF16)
            nc.vector.tensor_copy(iota_bft[:], iota_sb)
            iota_bf = iota_bft

        w_acc = outp.tile([128, NT, KTOP], F32)
        idx_acc = outp.tile([128, NT, KTOP], U32)

        loop_cm = tc.For_i(0, bench_iters, 1) if bench_iters else nullcontext()
        ctx.enter_context(loop_cm)

        xfix = None
        if mode == "gemmfix":
            # PE-isolation probe: one resident x chunk, no streaming DMA
            xfix = const.tile([128, KPG, 2, 512], F16)
            nc.scalar.dma_start(
                xfix[:].rearrange("p a b c -> p (a b c)"),
                xt_d[:, 0:KPG * 2 * 512],
            )

        def gemm_block(t0, tb, ob, hook=None):
            """Emit the GEMM for tokens [t0, t0+tb); returns psum accumulators.

            The tiny correction terms (xl*wh + xh*wl) accumulate in separate
            PSUM banks at their own scale instead of rounding at the main
            sum's ulp on every add; merged once in routing_block."""
            if stripe:
                # probe: stripe each matmul's 512-col output across the front
                # halves of TWO PSUM banks (cols 0-255 -> bank a, 256-511 ->
                # bank b) to test whether the 2-col/cycle stream mode (seen at
                # N<=256) engages when no single bank receives >1KB.
                psT0 = psTpool.tile([128, 2, tb], F32, tag="psT0")
                psT1 = psTpool.tile([128, 2, tb], F32, tag="psT1")
                psC0 = pspool.tile([128, 2, tb], F32, tag="psC0")
                psC1 = pspool.tile([128, 2, tb], F32, tag="psC1")
            elif concat:
                # [xh|xl] is contiguous in the x tile: one 1024-col matmul per
                # (k, half) computes main (left 512) and xl*wh (right 512) off
                # a single weight load; xh*wl is the remaining 512-col matmul.
                # Halves the Ldweights count (unmodeled in sim, real on HW).
                psT0 = psTpool.tile([128, 2, tb], F32, tag="psT0")
                psT1 = psTpool.tile([128, 2, tb], F32, tag="psT1")
            else:
                psT0 = psTpool.tile([128, tb], F32, tag="psT0")
                psT1 = psTpool.tile([128, tb], F32, tag="psT1")
            if not stripe:
                psC0 = pspool.tile([128, tb], F32, tag="psC0")
                psC1 = pspool.tile([128, tb], F32, tag="psC1")
            seg = KPG * 2 * tb
            for kg in range(NKG):
                if kg == hook_kg and hook is not None:
                    # emit the previous block's routing a few kg-chunks into
                    # this block's matmul stream: by the time the PE queue
                    # reaches the routing transposes, the DVE copies/folds
                    # they depend on have long finished -> no PE stall
                    hook()
                if xfix is not None:
                    xt = xfix
                else:
                    xt = xpool.tile([128, KPG, 2, tb], F16, tag="xt")
                    nc.sync.dma_start(
                        xt[:].rearrange("p a b c -> p (a b c)"),
                        xt_d[:, ob + kg * seg:ob + (kg + 1) * seg],
                    )
                for k2 in range(KPG):
                    k = kg * KPG + k2
                    first = (k == 0)
                    last = (k == KD - 1)
                    xh = xt[:, k2, 0, :]
                    xl = xt[:, k2, 1, :]
                    for h, psT, psC in ((0, psT0, psC0), (1, psT1, psC1)):
                        for _ in range(ldextra):
                            nc.tensor.ldweights(wt_sb[:, k, h, 0, :])
                        if concat:
                            nc.tensor.matmul(
                                psT[:].rearrange("p a b -> p (a b)"),
                                wt_sb[:, k, h, 0, :],
                                xt[:, k2, :, :].rearrange("p a b -> p (a b)"),
                                start=first, stop=last,
                            )
                            nc.tensor.matmul(
                                psC[:], wt_sb[:, k, h, 1, :], xh,
                                start=first, stop=last,
                            )
                        elif stripe:
                            oT = psT[:, :, 0:tb // 2]
                            oC = psC[:, :, 0:tb // 2]
                            nc.tensor.matmul(
                                oT, wt_sb[:, k, h, 0, :], xh, start=first, stop=last)
                            nc.tensor.matmul(
                                oC, wt_sb[:, k, h, 0, :], xl, start=first, stop=False)
                            nc.tensor.matmul(
                                oC, wt_sb[:, k, h, 1, :], xh, start=False, stop=last)
                        elif wpair:
                            # same weights feed two streams: probes whether
                            # the HW reloads identical weights per matmul
                            nc.tensor.matmul(
                                psT[:], wt_sb[:, k, h, 0, :], xh,
                                start=first, stop=last,
                            )
                            nc.tensor.matmul(
                                psC[:], wt_sb[:, k, h, 0, :], xl,
                                start=first, stop=last,
                            )
                        else:
                            nc.tensor.matmul(
                                psT[:], wt_sb[:, k, h, 0, :], xh,
                                start=first, stop=last,
                            )
                            if terms >= 2:
                                nc.tensor.matmul(
                                    psC[:], wt_sb[:, k, h, 0, :], xl,
                                    start=first, stop=(last and terms == 2),
                                )
                            if terms >= 3:
                                nc.tensor.matmul(
                                    psC[:], wt_sb[:, k, h, 1, :], xh,
                                    start=False, stop=last,
                                )
            return psT0, psT1, psC0, psC1

        def drain_slices(sT0, sT1, psT0, psT1, psC0, psC1, sl):
            # PSUM -> SBUF (PE transpose reads SBUF only), folding in the
            # correction accumulators (1 PSUM operand max per op).
            if actcopy:
                nc.scalar.activation(sT0[:, sl], psT0[:, sl], Act.Copy)
                nc.scalar.activation(sT1[:, sl], psT1[:, sl], Act.Copy)
            else:
                nc.vector.tensor_copy(sT0[:, sl], psT0[:, sl])
                nc.vector.tensor_copy(sT1[:, sl], psT1[:, sl])
            nc.vector.tensor_tensor(out=sT0[:, sl], in0=sT0[:, sl], in1=psC0[:, sl], op=Alu.add)
            nc.vector.tensor_tensor(out=sT1[:, sl], in0=sT1[:, sl], in1=psC1[:, sl], op=Alu.add)

        def routing_block(t0, tb, psT0, psT1, psC0, psC1):
            sT0 = work.tile([128, tb], F32, tag="sT0")
            sT1 = work.tile([128, tb], F32, tag="sT1")
            if concat:
                nc.vector.tensor_copy(sT0[:], psT0[:, 0, :])
                nc.vector.tensor_copy(sT1[:], psT1[:, 0, :])
                nc.vector.tensor_tensor(out=sT0[:], in0=sT0[:], in1=psT0[:, 1, :], op=Alu.add)
                nc.vector.tensor_tensor(out=sT1[:], in0=sT1[:], in1=psT1[:, 1, :], op=Alu.add)
                nc.vector.tensor_tensor(out=sT0[:], in0=sT0[:], in1=psC0[:], op=Alu.add)
                nc.vector.tensor_tensor(out=sT1[:], in0=sT1[:], in1=psC1[:], op=Alu.add)
            elif not qdrain:
                drain_slices(sT0, sT1, psT0, psT1, psC0, psC1, slice(0, tb))

            nq = tb // 128
            if trregion:
                # all of this block's transposes land in disjoint 128-col
                # regions of two banks (measured slower than ping-pong tiles
                # on uniform blocks; kept for experiments)
                tr0a = trpool.tile([128, nq, 128], F32, tag="tr0")
                tr1a = trpool.tile([128, nq, 128], F32, tag="tr1")
            for q in range(nq):
                t = t0 // 128 + q
                if qdrain and not concat:
                    drain_slices(sT0, sT1, psT0, psT1, psC0, psC1,
                                 slice(q * 128, (q + 1) * 128))
                if trregion:
                    tr0 = tr0a[:, q]
                    tr1 = tr1a[:, q]
                else:
                    trt = trpool.tile([128, 2, 128], F32, tag="trp")
                    tr0 = trt[:, 0]
                    tr1 = trt[:, 1]
                nc.tensor.transpose(tr0, sT0[:, q * 128:(q + 1) * 128], ident_sb)
                nc.tensor.transpose(tr1, sT1[:, q * 128:(q + 1) * 128], ident_sb)

                orig = work.tile([128, E], F32, tag="orig")
                # scores sit in the (x*2^8)(w*2^16) domain; 2^-24 descale is
                # exact (power of two) and free via the activation scale
                nc.scalar.activation(orig[:, 0:128], tr0, Act.Sigmoid, scale=SIGSCALE)
                nc.scalar.activation(orig[:, 128:E], tr1, Act.Sigmoid, scale=SIGSCALE)
                if bfgather:
                    # second, bf16 copy of the sigmoid for the output gathers
                    # (w tolerates bf16; the top-k chain stays fp32)
                    origb = work.tile([128, E], BF16, tag="origb")
                    nc.scalar.activation(origb[:, 0:128], tr0, Act.Sigmoid, scale=SIGSCALE)
                    nc.scalar.activation(origb[:, 128:E], tr1, Act.Sigmoid, scale=SIGSCALE)

                s = work.tile([128, E], F32, tag="s")
                nc.vector.tensor_add(s[:], orig[:], bias_sb)
                sg = s[:].rearrange("p (g f) -> p g f", g=G)

                m1 = small.tile([128, G], F32, tag="m1")
                nc.vector.reduce_max(m1[:], sg, axis=X)
                tmp = work.tile([128, E], F32, tag="tmp")
                nc.vector.match_replace(
                    out=tmp[:], in_to_replace=m1[:], in_values=s[:], imm_value=-BIG
                )
                m2 = small.tile([128, G], F32, tag="m2")
                nc.vector.reduce_max(
                    m2[:], tmp[:].rearrange("p (g f) -> p g f", g=G), axis=X
                )
                gs = small.tile([128, G], F32, tag="gs")
                nc.vector.tensor_add(gs[:], m1[:], m2[:])

                g8 = small.tile([128, 8], F32, tag="g8")
                nc.vector.max(out=g8[:], in_=gs[:])
                pen = small.tile([128, G], F32, tag="pen")
                nc.vector.tensor_scalar(
                    pen[:], gs[:], g8[:, 3:4], -BIG, op0=Alu.is_lt, op1=Alu.mult
                )

                masked = work.tile([128, E], F32, tag="masked")
                pen_b = pen[:].unsqueeze(2).broadcast_to([128, G, E // G])
                nc.vector.tensor_tensor(
                    out=masked[:].rearrange("p (g f) -> p g f", g=G),
                    in0=sg, in1=pen_b, op=Alu.add,
                )

                v8 = small.tile([128, KTOP], F32, tag="v8")
                nc.vector.max(out=v8[:], in_=masked[:])
                nc.vector.max_index(idx_acc[:, t, :], v8[:], masked[:])

                w8raw = small.tile([128, KTOP], F32, tag="w8raw")
                if bfgather:
                    idxf = small.tile([128, KTOP], BF16, tag="idxf")
                    nc.vector.tensor_copy(idxf[:], idx_acc[:, t, :])
                    scratch = work.tile([128, E], BF16, tag="scratch")
                    for j in range(KTOP):
                        nc.vector.scalar_tensor_tensor(
                            out=scratch[:], in0=iota_bf[:], scalar=idxf[:, j:j + 1],
                            in1=origb[:], op0=Alu.is_equal, op1=Alu.mult,
                            accum_out=w8raw[:, j:j + 1],
                        )
                else:
                    idxf = small.tile([128, KTOP], F32, tag="idxf")
                    nc.vector.tensor_copy(idxf[:], idx_acc[:, t, :])
                    scratch = work.tile([128, E], F32, tag="scratch")
                    for j in range(KTOP):
                        nc.vector.scalar_tensor_tensor(
                            out=scratch[:], in0=iota_sb, scalar=idxf[:, j:j + 1],
                            in1=orig[:], op0=Alu.is_equal, op1=Alu.mult,
                            accum_out=w8raw[:, j:j + 1],
                        )
                sum8 = small.tile([128, 1], F32, tag="sum8")
                nc.vector.reduce_sum(sum8[:], w8raw[:], axis=X)
                rec = small.tile([128, 1], F32, tag="rec")
                nc.vector.reciprocal(rec[:], sum8[:])
                nc.vector.tensor_scalar(
                    w_acc[:, t, :], w8raw[:], rec[:], ROUTE_SCALE,
                    op0=Alu.mult, op1=Alu.mult,
                )

        # pipeline=True: emit block b's GEMM, then block b-1's routing (PE
        # stream of b overlaps DVE routing of b-1, but routing's PE transposes
        # then queue behind ALL of block b's matmuls -> 2-deep lag and a long
        # serial tail). pipeline=False: routing emitted right after its own
        # GEMM; transposes cost a short PE bubble per block boundary but the
        # DVE routing of block b overlaps the GEMM of block b+1 with no lag.
        def dma_block(t0, tb, ob):
            seg = KPG * 2 * tb
            for kg in range(NKG):
                xt = xpool.tile([128, KPG, 2, tb], F16, tag="xt")
                nc.sync.dma_start(
                    xt[:].rearrange("p a b c -> p (a b c)"),
                    xt_d[:, ob + kg * seg:ob + (kg + 1) * seg],
                )

        def drain_block(t0, tb, psT0, psT1, psC0, psC1):
            sT0 = work.tile([128, tb], F32, tag="sT0")
            sT1 = work.tile([128, tb], F32, tag="sT1")
            if stripe:
                sv0 = sT0[:].rearrange("p (a b) -> p a b", a=2)
                sv1 = sT1[:].rearrange("p (a b) -> p a b", a=2)
                nc.vector.tensor_copy(sv0, psT0[:, :, 0:tb // 2])
                nc.vector.tensor_copy(sv1, psT1[:, :, 0:tb // 2])
                nc.vector.tensor_tensor(out=sv0, in0=sv0, in1=psC0[:, :, 0:tb // 2], op=Alu.add)
                nc.vector.tensor_tensor(out=sv1, in0=sv1, in1=psC1[:, :, 0:tb // 2], op=Alu.add)
                return
            if concat:
                nc.vector.tensor_copy(sT0[:], psT0[:, 0, :])
                nc.vector.tensor_copy(sT1[:], psT1[:, 0, :])
                nc.vector.tensor_tensor(out=sT0[:], in0=sT0[:], in1=psT0[:, 1, :], op=Alu.add)
                nc.vector.tensor_tensor(out=sT1[:], in0=sT1[:], in1=psT1[:, 1, :], op=Alu.add)
            else:
                nc.vector.tensor_copy(sT0[:], psT0[:])
                nc.vector.tensor_copy(sT1[:], psT1[:])
            if terms >= 2 or wpair:
                nc.vector.tensor_tensor(out=sT0[:], in0=sT0[:], in1=psC0[:], op=Alu.add)
                nc.vector.tensor_tensor(out=sT1[:], in0=sT1[:], in1=psC1[:], op=Alu.add)

        offs = np.cumsum([0] + blocks).tolist()
        for _rep in range(repeat):
            if mode == "dma":
                for b, tb in enumerate(blocks):
                    dma_block(offs[b], tb, KD * 2 * offs[b])
                continue
            if mode in ("gemm", "gemmfix"):
                for b, tb in enumerate(blocks):
                    ps = gemm_block(offs[b], tb, KD * 2 * offs[b])
                    drain_block(offs[b], tb, *ps)
                continue
            if dbuf:
                pending = None
                for b, tb in enumerate(blocks):
                    hook = None
                    if pending is not None:
                        args = pending
                        hook = lambda a=args: routing_block(*a)
                    ps = gemm_block(offs[b], tb, KD * 2 * offs[b], hook=hook)
                    pending = (offs[b], tb, *ps)
                routing_block(*pending)
            elif pipeline:
                pending = None
                for b, tb in enumerate(blocks):
                    ps = gemm_block(offs[b], tb, KD * 2 * offs[b])
                    if pending is not None:
                        routing_block(*pending)
                    pending = (offs[b], tb, *ps)
                routing_block(*pending)
            else:
                for b, tb in enumerate(blocks):
                    ps = gemm_block(offs[b], tb, KD * 2 * offs[b])
                    routing_block(offs[b], tb, *ps)

            nc.sync.dma_start(w_out_d[:], w_acc[:])
            nc.sync.dma_start(idx_out_d[:], idx_acc[:])

    nc.compile()
    if ldskip:
        # The bass compile pipeline adds a standalone InstLdweights before
        # every 2-byte matmul while leaving the matmul itself self-loading
        # (ins = [ifmap, weights]); on hardware the weights then load twice.
        # Drop every Ldweights that carries no semaphore waits/updates -- the
        # matmul's embedded load (same path fp32/f32r matmuls always use)
        # still provides the weights.
        ndrop = 0
        for blk in nc.m.functions[0].blocks:
            keep = []
            for inst in blk.instructions:
                if isinstance(inst, mybir.InstLdweights):
                    si = inst.sync_info
                    if si is None or (not si.on_wait and not si.on_update):
                        ndrop += 1
                        continue
                keep.append(inst)
            blk.instructions = keep
        assert ndrop > 0
    if ldskip2:
        # Walrus pairs each matmul with the most recent Ldweights; when two
        # consecutive Lds load the IDENTICAL weights AP, the second is
        # redundant (the PE weight registers still hold them). Drop it if it
        # carries no semaphore traffic.
        def _key(ld):
            ap = ld.ins[0]
            return (ap.memref, ap.offset, tuple(map(tuple, ap.ap)), ap.dtype)

        ndrop = 0
        for blk in nc.m.functions[0].blocks:
            keep = []
            last = None
            for inst in blk.instructions:
                if isinstance(inst, mybir.InstLdweights):
                    k = _key(inst)
                    si = inst.sync_info
                    free = si is None or (not si.on_wait and not si.on_update)
                    if free and last is not None and k == last:
                        ndrop += 1
                        continue
                    last = k
                elif isinstance(inst, mybir.InstMatmult) and inst.is_transpose:
                    # transposes are self-loading (identity) and clobber the
                    # PE weight registers
                    last = None
                keep.append(inst)
            blk.instructions = keep
        assert ndrop > 0, "ldskip2 found nothing to drop"
    return nc


def _prep_inputs(x, weight, bias):
    """Host-side sharding + layout transforms (all DMAs become contiguous)."""
    x = np.asarray(x, dtype=np.float32)
    weight = np.asarray(weight, dtype=np.float32)
    bias = np.asarray(bias, dtype=np.float32)

    # wt[p, k, h, hl, e'] = part[h*128+e', k*128+p], split in the w*2^16
    # domain so the fp16 lo part stays normal
    def to_tiles(wm):
        return wm.T.reshape(KD, 128, 2, 128).transpose(1, 0, 2, 3)
    ws = weight * np.float32(WSCALE)
    wh = ws.astype(np.float16)
    wl = (ws - wh.astype(np.float32)).astype(np.float16)
    wt = np.ascontiguousarray(
        np.stack([to_tiles(wh), to_tiles(wl)], axis=3)
    ).reshape(128, KD * E * 2)

    bias_b = np.broadcast_to(bias, (128, E))
    iota = np.broadcast_to(np.arange(E, dtype=np.float32), (128, E))
    ident = np.eye(128, dtype=np.float32)
    bi = np.ascontiguousarray(np.concatenate([bias_b, iota, ident], axis=1))

    offs = np.cumsum([0] + BLOCKS).tolist()
    in_maps = []
    for c in range(NCORES):
        xs = x[c * TCORE:(c + 1) * TCORE] * np.float32(XSCALE)
        # xk[p, k, t] = xs[t, k*128 + p]
        xk = xs.reshape(TCORE, KD, 128).transpose(2, 1, 0)
        xh = xk.astype(np.float16)
        xl = (xk - xh.astype(np.float32)).astype(np.float16)
        xfull = np.stack([xh, xl], axis=2)  # [p, k, 2, t]
        # block-major: per block a contiguous [k, 2, tb] segment
        segs = [
            xfull[:, :, :, offs[b]:offs[b + 1]].reshape(128, -1)
            for b in range(len(BLOCKS))
        ]
        xt = np.ascontiguousarray(np.concatenate(segs, axis=1))
        in_maps.append({"xt": xt, "wt": wt, "bi": bi})
    return in_maps


def _postprocess(results):
    ws, idxs = [], []
    for c in range(NCORES):
        w = results[c]["w_out"].reshape(128, NT, KTOP).transpose(1, 0, 2).reshape(TCORE, KTOP)
        ix = results[c]["idx_out"].reshape(128, NT, KTOP).transpose(1, 0, 2).reshape(TCORE, KTOP)
        ws.append(w)
        idxs.append(ix)
    w_full = np.concatenate(ws, axis=0).astype(np.float32)
    idx_full = np.concatenate(idxs, axis=0).astype(np.int32)
    return w_full, idx_full


def get_runner():
    """Build (once) and return a callable: in_maps -> per-core results list."""
    if "runner" in _CACHE:
        return _CACHE["runner"]

    from concourse.bass_utils import run_bass_kernel_spmd

    nc = _build()

    def runner(in_maps):
        return run_bass_kernel_spmd(nc, in_maps, list(range(NCORES))).results

    _CACHE["runner"] = runner
    _CACHE["nc"] = nc
    return runner


def kernel(x, weight, bias):
    runner = get_runner()
    in_maps = _prep_inputs(x, weight, bias)
    results = runner(in_maps)
    return _postprocess(results)


if __name__ == "__main__":
    rng = np.random.default_rng(0)
    x = rng.standard_normal((T, D), dtype=np.float32)
    w = rng.standard_normal((E, D), dtype=np.float32) * 0.02
    b = rng.standard_normal((E,), dtype=np.float32) * 0.02
    out_w, out_idx = kernel(x, w, b)
    print(out_w.shape, out_w.dtype, out_idx.shape, out_idx.dtype)
    print(out_w[0], out_idx[0])



# revision 28
# speedup vs baseline: 1.0870x; 1.0870x over previous
"""MoE gate routing kernel for Trainium2 (8 NeuronCores, SPMD token-parallel).

Problem: scores = sigmoid(x @ weight.T); s = scores + bias;
group top-2 sums -> top-4 groups mask -> global top-8 -> gather original
scores -> normalize * 2.5. Returns (w [T,8] f32, idx [T,8] int32).

Exact top-k agreement with the fp32 reference needs fp32-quality scores.
The GEMM runs as a 3-term decomposition at 1 cycle/column each:
    score = xh*wh + xl*wh + xh*wl
where BOTH operands are pre-split ON THE HOST into fp16 hi/lo pairs
(11+11 = 22 mantissa bits each; residuals 2^-22, below the fp32
reference's own einsum summation noise). The splits run in scaled
domains -- x*2^8 and w*2^16 -- so the lo parts stay out of the fp16
subnormal range; the combined 2^24 factor is undone exactly (power of
two) by the sigmoid activation's scale argument. The BIR verifier
requires matmul operand dtypes to match for fp32/f32r, so fp16 x fp16
is the only 1-cycle/column pairing that allows a host-side split.
Benefits vs the previous on-chip f32r-split design: the DVE no longer
spends ~165us splitting x on-chip (it only does routing, which hides
under the PE stream), x DMA bytes are unchanged (2 x fp16 = 1 x fp32),
and the weight DMA halves to 7.3MB.

Structure: weight-stationary [e,t] GEMM with tapered token blocks
[512,512,384,384,256] (the serial tail after the last GEMM is that
block's routing chain, so later blocks shrink), PSUM->SBUF drain copies
on the ACT engine (frees DVE ordering and releases PSUM accumulators
sooner), PE-transpose of score blocks back to [t,e], sigmoid on ACT
from PSUM, routing on DVE with max/max_index/match_replace.

Perf notes (HW-measured, difference-method timing):
  - per fp16 matmul (N=512): ~270ns = 213ns stream (1 col/cyc @2.4GHz)
    + ~55ns serial weight load; the load is NOT hidden by the reorder
    window and NOT skipped for identical back-to-back weights (probed).
  - 3-term GEMM alone = ~365-375us; DMA floor (58.7MB x as fp16 hi/lo
    pairs) = 172us; total = ~385-395us -> PE-bound.
  - matmul N>512 fp32-out fails the walrus ISA check (concat mode is
    dead on HW); striping one matmul's output across two PSUM banks
    does not unlock a faster stream mode (probed).
  - fp8 (e4m3) DoubleRow carrying both corrections in one matmul is
    ~1.45x faster on PE but loses idx-exactness (6/16384 mismatched
    tokens, w relerr 4.6e-2 > 2e-2 gate) -- rejected.
  - ldskip/ldskip2 (dropping standalone InstLdweights) have no effect;
    there is no double weight load on HW.
  - the ~20us above GEMM-only is fully accounted: ~9us PE transposes
    (structural PE cycles), ~9us exposed tail = last block's DVE routing
    chain (~3.5us per 128-token tile; taper minimizes it), ~3us head DMA
    latency. psT/psC double-buffering, hook positions, KPG=4 DMA
    granularity, and qdrain all measure as noise (+-5us run variance).
  - PSUM tile allocation is bank-granular (a 512B tile takes a 2KB
    bank); tr0/tr1 are packed into one [128,2,128] tile (1 bank).
  - tensor_tensor_reduce cannot fuse the bias-add with the per-group
    max (accum_out must be [128,1], not per-group).
"""
import sys

if "/opt/trn_rl_repo" not in sys.path:
    sys.path.insert(0, "/opt/trn_rl_repo")

import numpy as np

T, D, E = 16384, 7168, 256
G, KG, KTOP = 8, 4, 8
ROUTE_SCALE = 2.5
NCORES = 8
TCORE = T // NCORES          # 2048 tokens per core
KD = D // 128                # 56 contraction chunks
XSCALE = 256.0               # x pre-scale (2^8): keeps fp16 lo part normal
WSCALE = 65536.0             # w pre-scale (2^16): keeps fp16 lo part normal
SIGSCALE = 1.0 / (256.0 * 65536.0)   # exact 2^-24 descale at the sigmoid
# Token blocks (matmul N <= 512 = PSUM bank). Tapered: the exposed tail
# after the last GEMM block is that block's routing (DVE chain + PE
# transposes), so shrinking the LAST blocks cuts the serial tail, while
# keeping early blocks at 512 for stream efficiency. Measured: uniform
# 512x4 = 399.3us, this taper + ACT-engine drain copies = 384.7us.
# Blocks must be multiples of 128 (routing transposes are 128x128).
BLOCKS = [512, 512, 384, 384, 256]
assert sum(BLOCKS) == TCORE
NT = TCORE // 128            # 16 token tiles per core
KPG = 2                      # k-chunks per DMA
NKG = KD // KPG              # DMAs per block
NWSPLIT = 14                 # weight DMA split (shrinks head bubble)
BIG = 1e30

_CACHE = {}


def _build(bench_iters=0, pipeline=False, nwsplit=NWSPLIT, xbufs=4, blocks=None,
           repeat=1, mode="full", concat=False, trbufs=2, dbuf=False,
           hook_kg=2, ldskip=False, trregion=False, ldextra=0, ldskip2=False,
           terms=3, wpair=False, actcopy=True, qdrain=False, bfgather=False,
           stripe=False, dbuf2=False, kpg=None):
    import concourse.bacc as bacc
    import concourse.mybir as mybir
    import concourse.tile as tile
    from contextlib import ExitStack, nullcontext

    blocks = list(blocks or BLOCKS)
    assert sum(blocks) == TCORE
    KPG = kpg or globals()["KPG"]
    NKG = KD // KPG

    F32 = mybir.dt.float32
    F16 = mybir.dt.float16
    BF16 = mybir.dt.bfloat16
    U32 = mybir.dt.uint32
    X = mybir.AxisListType.X
    Alu = mybir.AluOpType
    Act = mybir.ActivationFunctionType

    nc = bacc.Bacc(None, target_bir_lowering=False, debug=False)

    # x pre-split on host, block-major so every (block, kg) DMA is one
    # contiguous span per partition: per block b the segment holds
    # [KD, {hi,lo}, tb] fp16 flattened
    xt_d = nc.dram_tensor("xt", [128, KD * 2 * TCORE], F16, kind="ExternalInput")
    # weights pre-split into (hi, lo) fp16 parts on the host
    wt_d = nc.dram_tensor("wt", [128, KD * E * 2], F16, kind="ExternalInput")
    bi_d = nc.dram_tensor("bi", [128, 2 * E + 128], F32, kind="ExternalInput")
    w_out_d = nc.dram_tensor("w_out", [128, NT * KTOP], F32, kind="ExternalOutput")
    idx_out_d = nc.dram_tensor("idx_out", [128, NT * KTOP], U32, kind="ExternalOutput")

    with tile.TileContext(nc) as tc, ExitStack() as ctx:
        const = ctx.enter_context(tc.tile_pool(name="const", bufs=1))
        outp = ctx.enter_context(tc.tile_pool(name="outp", bufs=1))
        xpool = ctx.enter_context(tc.tile_pool(name="xp", bufs=xbufs))
        # dbuf: double-buffer the psT accumulators (4 banks) so block b+1's
        # first matmuls never wait on block b's routing copies; psC single
        # (2 banks) + trpool single (2 banks) = 8 banks total.
        pstb = 2 if (dbuf or dbuf2) else 1
        if dbuf:
            trbufs = 1
        if dbuf2:
            dbuf = True
        psTpool = ctx.enter_context(tc.tile_pool(name="psT", bufs=pstb, space="PSUM"))
        pspool = ctx.enter_context(tc.tile_pool(name="ps", bufs=1, space="PSUM"))
        trpool = ctx.enter_context(tc.tile_pool(name="tr", bufs=trbufs, space="PSUM"))
        work = ctx.enter_context(tc.tile_pool(name="work", bufs=2))
        small = ctx.enter_context(tc.tile_pool(name="small", bufs=2))

        # wt_sb[p, k, e_half, hl, e']
        wt_sb = const.tile([128, KD, 2, 2, 128], F16)
        bi_sb = const.tile([128, 2 * E + 128], F32)
        # split the weight load (by k-chunk range) so the first matmuls only
        # wait on a fraction of the 7.3MB weight transfer; issue on the ACT
        # HWDGE queue so x-tile DMAs (SP queue) are not stuck behind it
        ksp = KD // nwsplit
        wt_dv = wt_d[:].rearrange("p (s r) -> p s r", s=nwsplit)
        for sp in range(nwsplit):
            nc.scalar.dma_start(wt_sb[:, sp * ksp:(sp + 1) * ksp], wt_dv[:, sp])
        nc.scalar.dma_start(bi_sb[:], bi_d[:])
        bias_sb = bi_sb[:, 0:E]
        iota_sb = bi_sb[:, E:2 * E]
        ident_sb = bi_sb[:, 2 * E:2 * E + 128]
        iota_bf = None
        if bfgather:
            # bf16 iota (0..255 exact in bf16) for 2x-throughput DVE gathers
            iota_bft = const.tile([128, E], BF16)
            nc.vector.tensor_copy(iota_bft[:], iota_sb)
            iota_bf = iota_bft

        w_acc = outp.tile([128, NT, KTOP], F32)
        idx_acc = outp.tile([128, NT, KTOP], U32)

        loop_cm = tc.For_i(0, bench_iters, 1) if bench_iters else nullcontext()
        ctx.enter_context(loop_cm)

        xfix = None
        if mode == "gemmfix":
            # PE-isolation probe: one resident x chunk, no streaming DMA
            xfix = const.tile([128, KPG, 2, 512], F16)
            nc.scalar.dma_start(
                xfix[:].rearrange("p a b c -> p (a b c)"),
                xt_d[:, 0:KPG * 2 * 512],
            )

        def gemm_block(t0, tb, ob, hook=None):
            """Emit the GEMM for tokens [t0, t0+tb); returns psum accumulators.

            The tiny correction terms (xl*wh + xh*wl) accumulate in separate
            PSUM banks at their own scale instead of rounding at the main
            sum's ulp on every add; merged once in routing_block."""
            if stripe:
                # probe: stripe each matmul's 512-col output across the front
                # halves of TWO PSUM banks (cols 0-255 -> bank a, 256-511 ->
                # bank b) to test whether the 2-col/cycle stream mode (seen at
                # N<=256) engages when no single bank receives >1KB.
                psT0 = psTpool.tile([128, 2, tb], F32, tag="psT0")
                psT1 = psTpool.tile([128, 2, tb], F32, tag="psT1")
                psC0 = pspool.tile([128, 2, tb], F32, tag="psC0")
                psC1 = pspool.tile([128, 2, tb], F32, tag="psC1")
            elif concat:
                # [xh|xl] is contiguous in the x tile: one 1024-col matmul per
                # (k, half) computes main (left 512) and xl*wh (right 512) off
                # a single weight load; xh*wl is the remaining 512-col matmul.
                # Halves the Ldweights count (unmodeled in sim, real on HW).
                psT0 = psTpool.tile([128, 2, tb], F32, tag="psT0")
                psT1 = psTpool.tile([128, 2, tb], F32, tag="psT1")
            else:
                psT0 = psTpool.tile([128, tb], F32, tag="psT0")
                psT1 = psTpool.tile([128, tb], F32, tag="psT1")
            if not stripe:
                psC0 = pspool.tile([128, tb], F32, tag="psC0")
                psC1 = pspool.tile([128, tb], F32, tag="psC1")
            seg = KPG * 2 * tb
            for kg in range(NKG):
                if kg == hook_kg and hook is not None:
                    # emit the previous block's routing a few kg-chunks into
                    # this block's matmul stream: by the time the PE queue
                    # reaches the routing transposes, the DVE copies/folds
                    # they depend on have long finished -> no PE stall
                    hook()
                if xfix is not None:
                    xt = xfix
                else:
                    xt = xpool.tile([128, KPG, 2, tb], F16, tag="xt")
                    nc.sync.dma_start(
                        xt[:].rearrange("p a b c -> p (a b c)"),
                        xt_d[:, ob + kg * seg:ob + (kg + 1) * seg],
                    )
                for k2 in range(KPG):
                    k = kg * KPG + k2
                    first = (k == 0)
                    last = (k == KD - 1)
                    xh = xt[:, k2, 0, :]
                    xl = xt[:, k2, 1, :]
                    for h, psT, psC in ((0, psT0, psC0), (1, psT1, psC1)):
                        for _ in range(ldextra):
                            nc.tensor.ldweights(wt_sb[:, k, h, 0, :])
                        if concat:
                            nc.tensor.matmul(
                                psT[:].rearrange("p a b -> p (a b)"),
                                wt_sb[:, k, h, 0, :],
                                xt[:, k2, :, :].rearrange("p a b -> p (a b)"),
                                start=first, stop=last,
                            )
                            nc.tensor.matmul(
                                psC[:], wt_sb[:, k, h, 1, :], xh,
                                start=first, stop=last,
                            )
                        elif stripe:
                            oT = psT[:, :, 0:tb // 2]
                            oC = psC[:, :, 0:tb // 2]
                            nc.tensor.matmul(
                                oT, wt_sb[:, k, h, 0, :], xh, start=first, stop=last)
                            nc.tensor.matmul(
                                oC, wt_sb[:, k, h, 0, :], xl, start=first, stop=False)
                            nc.tensor.matmul(
                                oC, wt_sb[:, k, h, 1, :], xh, start=False, stop=last)
                        elif wpair:
                            # same weights feed two streams: probes whether
                            # the HW reloads identical weights per matmul
                            nc.tensor.matmul(
                                psT[:], wt_sb[:, k, h, 0, :], xh,
                                start=first, stop=last,
                            )
                            nc.tensor.matmul(
                                psC[:], wt_sb[:, k, h, 0, :], xl,
                                start=first, stop=last,
                            )
                        else:
                            nc.tensor.matmul(
                                psT[:], wt_sb[:, k, h, 0, :], xh,
                                start=first, stop=last,
                            )
                            if terms >= 2:
                                nc.tensor.matmul(
                                    psC[:], wt_sb[:, k, h, 0, :], xl,
                                    start=first, stop=(last and terms == 2),
                                )
                            if terms >= 3:
                                nc.tensor.matmul(
                                    psC[:], wt_sb[:, k, h, 1, :], xh,
                                    start=False, stop=last,
                                )
            return psT0, psT1, psC0, psC1

        def drain_slices(sT0, sT1, psT0, psT1, psC0, psC1, sl):
            # PSUM -> SBUF (PE transpose reads SBUF only), folding in the
            # correction accumulators (1 PSUM operand max per op).
            if actcopy:
                nc.scalar.activation(sT0[:, sl], psT0[:, sl], Act.Copy)
                nc.scalar.activation(sT1[:, sl], psT1[:, sl], Act.Copy)
            else:
                nc.vector.tensor_copy(sT0[:, sl], psT0[:, sl])
                nc.vector.tensor_copy(sT1[:, sl], psT1[:, sl])
            nc.vector.tensor_tensor(out=sT0[:, sl], in0=sT0[:, sl], in1=psC0[:, sl], op=Alu.add)
            nc.vector.tensor_tensor(out=sT1[:, sl], in0=sT1[:, sl], in1=psC1[:, sl], op=Alu.add)

        def routing_block(t0, tb, psT0, psT1, psC0, psC1):
            sT0 = work.tile([128, tb], F32, tag="sT0")
            sT1 = work.tile([128, tb], F32, tag="sT1")
            if concat:
                nc.vector.tensor_copy(sT0[:], psT0[:, 0, :])
                nc.vector.tensor_copy(sT1[:], psT1[:, 0, :])
                nc.vector.tensor_tensor(out=sT0[:], in0=sT0[:], in1=psT0[:, 1, :], op=Alu.add)
                nc.vector.tensor_tensor(out=sT1[:], in0=sT1[:], in1=psT1[:, 1, :], op=Alu.add)
                nc.vector.tensor_tensor(out=sT0[:], in0=sT0[:], in1=psC0[:], op=Alu.add)
                nc.vector.tensor_tensor(out=sT1[:], in0=sT1[:], in1=psC1[:], op=Alu.add)
            elif not qdrain:
                drain_slices(sT0, sT1, psT0, psT1, psC0, psC1, slice(0, tb))

            nq = tb // 128
            if trregion:
                # all of this block's transposes land in disjoint 128-col
                # regions of two banks (measured slower than ping-pong tiles
                # on uniform blocks; kept for experiments)
                tr0a = trpool.tile([128, nq, 128], F32, tag="tr0")
                tr1a = trpool.tile([128, nq, 128], F32, tag="tr1")
            for q in range(nq):
                t = t0 // 128 + q
                if qdrain and not concat:
                    drain_slices(sT0, sT1, psT0, psT1, psC0, psC1,
                                 slice(q * 128, (q + 1) * 128))
                if trregion:
                    tr0 = tr0a[:, q]
                    tr1 = tr1a[:, q]
                else:
                    trt = trpool.tile([128, 2, 128], F32, tag="trp")
                    tr0 = trt[:, 0]
                    tr1 = trt[:, 1]
                nc.tensor.transpose(tr0, sT0[:, q * 128:(q + 1) * 128], ident_sb)
                nc.tensor.transpose(tr1, sT1[:, q * 128:(q + 1) * 128], ident_sb)

                orig = work.tile([128, E], F32, tag="orig")
                # scores sit in the (x*2^8)(w*2^16) domain; 2^-24 descale is
                # exact (power of two) and free via the activation scale
                nc.scalar.activation(orig[:, 0:128], tr0, Act.Sigmoid, scale=SIGSCALE)
                nc.scalar.activation(orig[:, 128:E], tr1, Act.Sigmoid, scale=SIGSCALE)
                if bfgather:
                    # second, bf16 copy of the sigmoid for the output gathers
                    # (w tolerates bf16; the top-k chain stays fp32)
                    origb = work.tile([128, E], BF16, tag="origb")
                    nc.scalar.activation(origb[:, 0:128], tr0, Act.Sigmoid, scale=SIGSCALE)
                    nc.scalar.activation(origb[:, 128:E], tr1, Act.Sigmoid, scale=SIGSCALE)

                s = work.tile([128, E], F32, tag="s")
                nc.vector.tensor_add(s[:], orig[:], bias_sb)
                sg = s[:].rearrange("p (g f) -> p g f", g=G)

                m1 = small.tile([128, G], F32, tag="m1")
                nc.vector.reduce_max(m1[:], sg, axis=X)
                tmp = work.tile([128, E], F32, tag="tmp")
                nc.vector.match_replace(
                    out=tmp[:], in_to_replace=m1[:], in_values=s[:], imm_value=-BIG
                )
                m2 = small.tile([128, G], F32, tag="m2")
                nc.vector.reduce_max(
                    m2[:], tmp[:].rearrange("p (g f) -> p g f", g=G), axis=X
                )
                gs = small.tile([128, G], F32, tag="gs")
                nc.vector.tensor_add(gs[:], m1[:], m2[:])

                g8 = small.tile([128, 8], F32, tag="g8")
                nc.vector.max(out=g8[:], in_=gs[:])
                pen = small.tile([128, G], F32, tag="pen")
                nc.vector.tensor_scalar(
                    pen[:], gs[:], g8[:, 3:4], -BIG, op0=Alu.is_lt, op1=Alu.mult
                )

                masked = work.tile([128, E], F32, tag="masked")
                pen_b = pen[:].unsqueeze(2).broadcast_to([128, G, E // G])
                nc.vector.tensor_tensor(
                    out=masked[:].rearrange("p (g f) -> p g f", g=G),
                    in0=sg, in1=pen_b, op=Alu.add,
                )

                v8 = small.tile([128, KTOP], F32, tag="v8")
                nc.vector.max(out=v8[:], in_=masked[:])
                nc.vector.max_index(idx_acc[:, t, :], v8[:], masked[:])

                w8raw = small.tile([128, KTOP], F32, tag="w8raw")
                if bfgather:
                    idxf = small.tile([128, KTOP], BF16, tag="idxf")
                    nc.vector.tensor_copy(idxf[:], idx_acc[:, t, :])
                    scratch = work.tile([128, E], BF16, tag="scratch")
                    for j in range(KTOP):
                        nc.vector.scalar_tensor_tensor(
                            out=scratch[:], in0=iota_bf[:], scalar=idxf[:, j:j + 1],
                            in1=origb[:], op0=Alu.is_equal, op1=Alu.mult,
                            accum_out=w8raw[:, j:j + 1],
                        )
                else:
                    idxf = small.tile([128, KTOP], F32, tag="idxf")
                    nc.vector.tensor_copy(idxf[:], idx_acc[:, t, :])
                    scratch = work.tile([128, E], F32, tag="scratch")
                    for j in range(KTOP):
                        nc.vector.scalar_tensor_tensor(
                            out=scratch[:], in0=iota_sb, scalar=idxf[:, j:j + 1],
                            in1=orig[:], op0=Alu.is_equal, op1=Alu.mult,
                            accum_out=w8raw[:, j:j + 1],
                        )
                sum8 = small.tile([128, 1], F32, tag="sum8")
                nc.vector.reduce_sum(sum8[:], w8raw[:], axis=X)
                rec = small.tile([128, 1], F32, tag="rec")
                nc.vector.reciprocal(rec[:], sum8[:])
                nc.vector.tensor_scalar(
                    w_acc[:, t, :], w8raw[:], rec[:], ROUTE_SCALE,
                    op0=Alu.mult, op1=Alu.mult,
                )

        # pipeline=True: emit block b's GEMM, then block b-1's routing (PE
        # stream of b overlaps DVE routing of b-1, but routing's PE transposes
        # then queue behind ALL of block b's matmuls -> 2-deep lag and a long
        # serial tail). pipeline=False: routing emitted right after its own
        # GEMM; transposes cost a short PE bubble per block boundary but the
        # DVE routing of block b overlaps the GEMM of block b+1 with no lag.
        def dma_block(t0, tb, ob):
            seg = KPG * 2 * tb
            for kg in range(NKG):
                xt = xpool.tile([128, KPG, 2, tb], F16, tag="xt")
                nc.sync.dma_start(
                    xt[:].rearrange("p a b c -> p (a b c)"),
                    xt_d[:, ob + kg * seg:ob + (kg + 1) * seg],
                )

        def drain_block(t0, tb, psT0, psT1, psC0, psC1):
            sT0 = work.tile([128, tb], F32, tag="sT0")
            sT1 = work.tile([128, tb], F32, tag="sT1")
            if stripe:
                sv0 = sT0[:].rearrange("p (a b) -> p a b", a=2)
                sv1 = sT1[:].rearrange("p (a b) -> p a b", a=2)
                nc.vector.tensor_copy(sv0, psT0[:, :, 0:tb // 2])
                nc.vector.tensor_copy(sv1, psT1[:, :, 0:tb // 2])
                nc.vector.tensor_tensor(out=sv0, in0=sv0, in1=psC0[:, :, 0:tb // 2], op=Alu.add)
                nc.vector.tensor_tensor(out=sv1, in0=sv1, in1=psC1[:, :, 0:tb // 2], op=Alu.add)
                return
            if concat:
                nc.vector.tensor_copy(sT0[:], psT0[:, 0, :])
                nc.vector.tensor_copy(sT1[:], psT1[:, 0, :])
                nc.vector.tensor_tensor(out=sT0[:], in0=sT0[:], in1=psT0[:, 1, :], op=Alu.add)
                nc.vector.tensor_tensor(out=sT1[:], in0=sT1[:], in1=psT1[:, 1, :], op=Alu.add)
            else:
                nc.vector.tensor_copy(sT0[:], psT0[:])
                nc.vector.tensor_copy(sT1[:], psT1[:])
            if terms >= 2 or wpair:
                nc.vector.tensor_tensor(out=sT0[:], in0=sT0[:], in1=psC0[:], op=Alu.add)
                nc.vector.tensor_tensor(out=sT1[:], in0=sT1[:], in1=psC1[:], op=Alu.add)

        offs = np.cumsum([0] + blocks).tolist()
        for _rep in range(repeat):
            if mode == "dma":
                for b, tb in enumerate(blocks):
                    dma_block(offs[b], tb, KD * 2 * offs[b])
                continue
            if mode in ("gemm", "gemmfix"):
                for b, tb in enumerate(blocks):
                    ps = gemm_block(offs[b], tb, KD * 2 * offs[b])
                    drain_block(offs[b], tb, *ps)
                continue
            if dbuf:
                pending = None
                for b, tb in enumerate(blocks):
                    hook = None
                    if pending is not None:
                        args = pending
                        hook = lambda a=args: routing_block(*a)
                    ps = gemm_block(offs[b], tb, KD * 2 * offs[b], hook=hook)
                    pending = (offs[b], tb, *ps)
                routing_block(*pending)
            elif pipeline:
                pending = None
                for b, tb in enumerate(blocks):
                    ps = gemm_block(offs[b], tb, KD * 2 * offs[b])
                    if pending is not None:
                        routing_block(*pending)
                    pending = (offs[b], tb, *ps)
                routing_block(*pending)
            else:
                for b, tb in enumerate(blocks):
                    ps = gemm_block(offs[b], tb, KD * 2 * offs[b])
                    routing_block(offs[b], tb, *ps)

            nc.sync.dma_start(w_out_d[:], w_acc[:])
            nc.sync.dma_start(idx_out_d[:], idx_acc[:])

    nc.compile()
    if ldskip:
        # The bass compile pipeline adds a standalone InstLdweights before
        # every 2-byte matmul while leaving the matmul itself self-loading
        # (ins = [ifmap, weights]); on hardware the weights then load twice.
        # Drop every Ldweights that carries no semaphore waits/updates -- the
        # matmul's embedded load (same path fp32/f32r matmuls always use)
        # still provides the weights.
        ndrop = 0
        for blk in nc.m.functions[0].blocks:
            keep = []
            for inst in blk.instructions:
                if isinstance(inst, mybir.InstLdweights):
                    si = inst.sync_info
                    if si is None or (not si.on_wait and not si.on_update):
                        ndrop += 1
                        continue
                keep.append(inst)
            blk.instructions = keep
        assert ndrop > 0
    if ldskip2:
        # Walrus pairs each matmul with the most recent Ldweights; when two
        # consecutive Lds load the IDENTICAL weights AP, the second is
        # redundant (the PE weight registers still hold them). Drop it if it
        # carries no semaphore traffic.
        def _key(ld):
            ap = ld.ins[0]
            return (ap.memref, ap.offset, tuple(map(tuple, ap.ap)), ap.dtype)

        ndrop = 0
        for blk in nc.m.functions[0].blocks:
            keep = []
            last = None
            for inst in blk.instructions:
                if isinstance(inst, mybir.InstLdweights):
                    k = _key(inst)
                    si = inst.sync_info
                    free = si is None or (not si.on_wait and not si.on_update)
                    if free and last is not None and k == last:
                        ndrop += 1
                        continue
                    last = k
                elif isinstance(inst, mybir.InstMatmult) and inst.is_transpose:
                    # transposes are self-loading (identity) and clobber the
                    # PE weight registers
                    last = None
                keep.append(inst)
            blk.instructions = keep
        assert ndrop > 0, "ldskip2 found nothing to drop"
    return nc


def _prep_inputs(x, weight, bias):
    """Host-side sharding + layout transforms (all DMAs become contiguous)."""
    x = np.asarray(x, dtype=np.float32)
    weight = np.asarray(weight, dtype=np.float32)
    bias = np.asarray(bias, dtype=np.float32)

    # wt[p, k, h, hl, e'] = part[h*128+e', k*128+p], split in the w*2^16
    # domain so the fp16 lo part stays normal
    def to_tiles(wm):
        return wm.T.reshape(KD, 128, 2, 128).transpose(1, 0, 2, 3)
    ws = weight * np.float32(WSCALE)
    wh = ws.astype(np.float16)
    wl = (ws - wh.astype(np.float32)).astype(np.float16)
    wt = np.ascontiguousarray(
        np.stack([to_tiles(wh), to_tiles(wl)], axis=3)
    ).reshape(128, KD * E * 2)

    bias_b = np.broadcast_to(bias, (128, E))
    iota = np.broadcast_to(np.arange(E, dtype=np.float32), (128, E))
    ident = np.eye(128, dtype=np.float32)
    bi = np.ascontiguousarray(np.concatenate([bias_b, iota, ident], axis=1))

    offs = np.cumsum([0] + BLOCKS).tolist()
    in_maps = []
    for c in range(NCORES):
        xs = x[c * TCORE:(c + 1) * TCORE] * np.float32(XSCALE)
        # xk[p, k, t] = xs[t, k*128 + p]
        xk = xs.reshape(TCORE, KD, 128).transpose(2, 1, 0)
        xh = xk.astype(np.float16)
        xl = (xk - xh.astype(np.float32)).astype(np.float16)
        xfull = np.stack([xh, xl], axis=2)  # [p, k, 2, t]
        # block-major: per block a contiguous [k, 2, tb] segment
        segs = [
            xfull[:, :, :, offs[b]:offs[b + 1]].reshape(128, -1)
            for b in range(len(BLOCKS))
        ]
        xt = np.ascontiguousarray(np.concatenate(segs, axis=1))
        in_maps.append({"xt": xt, "wt": wt, "bi": bi})
    return in_maps


def _postprocess(results):
    ws, idxs = [], []
    for c in range(NCORES):
        w = results[c]["w_out"].reshape(128, NT, KTOP).transpose(1, 0, 2).reshape(TCORE, KTOP)
        ix = results[c]["idx_out"].reshape(128, NT, KTOP).transpose(1, 0, 2).reshape(TCORE, KTOP)
        ws.append(w)
        idxs.append(ix)
    w_full = np.concatenate(ws, axis=0).astype(np.float32)
    idx_full = np.concatenate(idxs, axis=0).astype(np.int32)
    return w_full, idx_full


def get_runner():
    """Build (once) and return a callable: in_maps -> per-core results list."""
    if "runner" in _CACHE:
        return _CACHE["runner"]

    from concourse.bass_utils import run_bass_kernel_spmd

    nc = _build()

    def runner(in_maps):
        return run_bass_kernel_spmd(nc, in_maps, list(range(NCORES))).results

    _CACHE["runner"] = runner
    _CACHE["nc"] = nc
    return runner


def kernel(x, weight, bias):
    runner = get_runner()
    in_maps = _prep_inputs(x, weight, bias)
    results = runner(in_maps)
    return _postprocess(results)


if __name__ == "__main__":
    rng = np.random.default_rng(0)
    x = rng.standard_normal((T, D), dtype=np.float32)
    w = rng.standard_normal((E, D), dtype=np.float32) * 0.02
    b = rng.standard_normal((E,), dtype=np.float32) * 0.02
    out_w, out_idx = kernel(x, w, b)
    print(out_w.shape, out_w.dtype, out_idx.shape, out_idx.dtype)
    print(out_w[0], out_idx[0])

